# revision 12
# baseline (speedup 1.0000x reference)
"""Causal self-attention (S=8192, D=2048, DKQ=DV=128, fp32) on 8 Trainium2 cores.

Strategy (sequence-parallel, causal-balanced):
- 64 query tiles of 128 rows. Core c owns 8 tiles: for pair p in 0..3 it gets
  global tiles gA = 8p + c (few key columns) and gB = 63 - 8p - c (many), so
  every core does identical work (one compiled program, SPMD).
- Precision: the PE's fp32/fp32r matmuls carry only ~13 mantissa bits and a
  score error d becomes an exp() relative error ~d with scores of O(100).
  So Q/K and the scores use an exact fp16 hi/lo 3-term split
  (hi*hi + hi*lo + lo*hi, fp32 PSUM accumulation), 1 cyc/row each.
  V, exp'd weights A, and PV run in bf16.
- Sequence tiles are laid out in "position" order with send permutation
  [0,2,4,6,7,5,3,1] per rank so the post-AllGather reorder of K^T/V into
  global column order is a handful of large affine DMAs (no negative
  strides). Position->tile map: pos<32: g=pos; else g=39+8q'-rr with
  q'=(pos-32)//8, rr=pos%8. Host masks encode causality per position.
- Phase C per slot: 3 fp16 score matmuls per 512-chunk -> PSUM; ACT copies
  chunks PSUM->SBUF while DVE maxes them (last 2 chunks: DVE adds host
  mask); one big ACT exp (bias=-rowmax*scale, accum=rowsum) emits bf16 A;
  PE transposes A (bf16, 4 per PSUM tile), DVE copies to SBUF, bf16 PV
  accumulates O[q,dv]; DVE applies 1/rowsum; PV of slot s-1 is emitted
  after the scores of slot s to keep the PE busy.
- DMA issue is spread across the SP and ACT hardware DGE queues.
"""

import os
import sys

for _p in ("/opt/trn_rl_repo", "/root/.axon_site/_ro/trn_rl_repo"):
    if os.path.isdir(_p) and _p not in sys.path:
        sys.path.append(_p)

import numpy as np

import concourse.bass as bass
import concourse.mybir as mybir
import concourse.tile as tile
from concourse import bacc
from concourse.bass_utils import run_bass_kernel_spmd
from concourse.masks import make_identity

P = 128
S = 8192
D = 2048
DK = 128
DV = 128
NCORES = 8
NSLOT = 8
MB = NSLOT * P  # rows per core
SCALE = 1.0 / float(np.sqrt(128.0))
NEG = -1.0e30
# slot s in pair p = s//2; even (A) slots compute 2p+2 score chunks of 512
# columns, odd (B) slots 16-2p.
C_SLOT = [2, 16, 4, 14, 6, 12, 8, 10]
PORDER = [0, 2, 4, 6, 7, 5, 3, 1]  # slot stored at local position j
POS = {s: j for j, s in enumerate(PORDER)}

f32 = mybir.dt.float32
bf16 = mybir.dt.bfloat16
fp16 = mybir.dt.float16


def _slot_to_g(c, s):
    p = s // 2
    return 8 * p + c if s % 2 == 0 else 63 - 8 * p - c


def _pos_to_g(pos):
    """Global column-block position -> global row-tile index it holds."""
    if pos < 32:
        return pos
    qp, rr = (pos - 32) // 8, pos % 8
    return 39 + 8 * qp - rr


def _build_nc():
    nc = bacc.Bacc(
        "TRN2", target_bir_lowering=False, debug=False, num_devices=NCORES
    )
    xt_hi = nc.dram_tensor("xt_hi", [P, 16, MB], fp16, kind="ExternalInput").ap()
    xt_lo = nc.dram_tensor("xt_lo", [P, 16, MB], fp16, kind="ExternalInput").ap()
    msk = nc.dram_tensor("mask", [NSLOT, 2, P, 512], bf16, kind="ExternalInput").ap()
    wqs = {}
    for nm in ("wq_hi", "wq_lo", "wk_hi", "wk_lo", "wv_hi"):
        wqs[nm] = nc.dram_tensor(nm, [D, DK], fp16, kind="ExternalInput").ap()
    out = nc.dram_tensor("out", [MB, DV], f32, kind="ExternalOutput").ap()

    cc_k_in = nc.dram_tensor("cc_k_in", [P, 2 * MB], fp16)
    cc_k_out = nc.dram_tensor(
        "cc_k_out", [NCORES * P, 2 * MB], fp16, addr_space="Shared"
    )
    cc_v_in = nc.dram_tensor("cc_v_in", [P, NSLOT * DV], bf16)
    cc_v_out = nc.dram_tensor(
        "cc_v_out", [NCORES * P, NSLOT * DV], bf16, addr_space="Shared"
    )

    AX = mybir.AxisListType
    OP = mybir.AluOpType
    ACT = mybir.ActivationFunctionType

    with tile.TileContext(nc) as tc:
        with (
            tc.tile_pool(name="const", bufs=1) as const_pool,
            tc.tile_pool(name="resident", bufs=1) as res_pool,
        ):
            ident = const_pool.tile([P, P], bf16)
            make_identity(nc, ident[:])

            q_hi = res_pool.tile([P, MB], fp16)  # Q^T hi, position-ordered
            q_lo = res_pool.tile([P, MB], fp16)
            kt_hi = res_pool.tile([P, S], fp16)  # full K^T hi
            kt_lo = res_pool.tile([P, S], fp16)
            vnat = res_pool.tile([P, 64, DV], bf16)  # V, 64 [kc,dv] tiles
            rinv = res_pool.tile([P, NSLOT], f32)  # per-slot 1/rowsum
            mask_sb = res_pool.tile([P, NSLOT, 2, 512], bf16)

            # ---------- phase A: projections of my 1024 rows ----------
            with (
                tc.tile_pool(name="pa_x", bufs=1) as pa_x,
                tc.tile_pool(name="pa_w", bufs=1) as pa_w,
                tc.tile_pool(name="pa_keep", bufs=1) as pa_keep,
                tc.tile_pool(name="pa_ps", bufs=2, space="PSUM") as pa_ps,
            ):
                xh = pa_x.tile([P, 16, MB], fp16, tag="xh")
                xl = pa_x.tile([P, 16, MB], fp16, tag="xl")
                nc.sync.dma_start(out=xh[:, :8], in_=xt_hi[:, :8])
                nc.scalar.dma_start(out=xh[:, 8:], in_=xt_hi[:, 8:])
                nc.sync.dma_start(out=xl[:, :8], in_=xt_lo[:, :8])
                nc.scalar.dma_start(out=xl[:, 8:], in_=xt_lo[:, 8:])
                w_sb = {}
                for name in ("wk_hi", "wk_lo", "wv_hi", "wq_hi", "wq_lo"):
                    wt = pa_w.tile([P, 16, DK], fp16, tag=name)
                    nc.gpsimd.dma_start(
                        out=wt[:],
                        in_=wqs[name].rearrange("(t p) d -> p t d", p=P),
                    )
                    w_sb[name] = wt
                vt_mine = pa_keep.tile([P, MB], bf16, tag="vtm")
                vnat_mine = pa_keep.tile([P, NSLOT, DV], bf16, tag="vnm")
                k_hi_mine = pa_keep.tile([P, MB], fp16, tag="khm")
                k_lo_mine = pa_keep.tile([P, MB], fp16, tag="klm")

                rg = [list(range(NCORES))]

                def proj3(ps, w_pfx, m2):
                    terms = [
                        (w_sb[w_pfx + "_hi"], xh),
                        (w_sb[w_pfx + "_lo"], xh),
                        (w_sb[w_pfx + "_hi"], xl),
                    ]
                    for ti, (wt, xop) in enumerate(terms):
                        for kk in range(16):
                            nc.tensor.matmul(
                                ps[:],
                                wt[:, kk, :],
                                xop[:, kk, bass.ts(m2, 512)],
                                start=(ti == 0 and kk == 0),
                                stop=(ti == 2 and kk == 15),
                            )

                # K first: its AllGather gates phase C
                for m2 in range(2):
                    kp = pa_ps.tile([P, 512], f32, tag="kp")
                    proj3(kp, "wk", m2)
                    sl = bass.ts(m2, 512)
                    nc.vector.tensor_copy(k_hi_mine[:, sl], kp[:])
                    nc.vector.tensor_sub(
                        k_lo_mine[:, sl], kp[:], k_hi_mine[:, sl]
                    )
                nc.sync.dma_start(out=cc_k_in[:, :MB], in_=k_hi_mine[:])
                nc.scalar.dma_start(out=cc_k_in[:, MB:], in_=k_lo_mine[:])
                # masks are only needed in phase C; SWDGE queue keeps the
                # HWDGE queues free for the gather reorder loads
                nc.gpsimd.dma_start(
                    out=mask_sb[:], in_=msk.rearrange("s two p c -> p s two c")
                )
                nc.gpsimd.collective_compute(
                    "AllGather",
                    OP.bypass,
                    replica_groups=rg,
                    ins=[cc_k_in[:]],
                    outs=[cc_k_out[:]],
                )

                # V next (bf16, transposed to natural layout), AllGather
                for m2 in range(2):
                    vp = pa_ps.tile([P, 512], f32, tag="vp")
                    for kk in range(16):
                        nc.tensor.matmul(
                            vp[:],
                            w_sb["wv_hi"][:, kk, :],
                            xh[:, kk, bass.ts(m2, 512)],
                            start=(kk == 0),
                            stop=(kk == 15),
                        )
                    nc.vector.tensor_copy(vt_mine[:, bass.ts(m2, 512)], vp[:])
                for t in range(NSLOT):
                    vtp = pa_ps.tile([P, P], bf16, tag="vtp")
                    nc.tensor.transpose(
                        vtp[:], vt_mine[:, bass.ts(t, P)], ident[:]
                    )
                    nc.vector.tensor_copy(vnat_mine[:, t, :], vtp[:])
                nc.sync.dma_start(
                    out=cc_v_in.rearrange("p (t d) -> p t d", d=DV),
                    in_=vnat_mine[:],
                )
                nc.gpsimd.collective_compute(
                    "AllGather",
                    OP.bypass,
                    replica_groups=rg,
                    ins=[cc_v_in[:]],
                    outs=[cc_v_out[:]],
                )

                # Q last, overlapping the collectives
                for m2 in range(2):
                    qp = pa_ps.tile([P, 512], f32, tag="qp")
                    proj3(qp, "wq", m2)
                    sl = bass.ts(m2, 512)
                    nc.vector.tensor_copy(q_hi[:, sl], qp[:])
                    nc.vector.tensor_sub(q_lo[:, sl], qp[:], q_hi[:, sl])

            # ---------- phase B: load gathered K^T and V ----------
            # cc_k_out row r*128+p holds rank r; its column block b (of 128)
            # holds slot PORDER[b] (hi half; +MB for lo). Position maps:
            #   pos 8q+r      <- rank r, send block q       (q = 0..3)
            #   pos 32+8q'+rr <- rank rr, send block 4+q'   (q' = 0..3)
            # Both ascending in every index -> affine DMAs, one per (q, half).
            for half, kt_dst in ((0, kt_hi), (1, kt_lo)):
                o = half * MB
                ktA = kt_dst[:, :4096].rearrange(
                    "p (q rb c) -> p q rb c", q=4, c=P
                )
                ktB = kt_dst[:, 4096:].rearrange(
                    "p (q rb c) -> p q rb c", q=4, c=P
                )
                for r in range(NCORES):
                    blk = cc_k_out[r * P : (r + 1) * P, o : o + MB]
                    eng = nc.sync if r % 2 == 0 else nc.scalar
                    eng.dma_start(
                        out=ktA[:, :, r],
                        in_=blk[:, :512].rearrange("p (q c) -> p q c", c=P),
                    )
                    eng = nc.scalar if r % 2 == 0 else nc.sync
                    eng.dma_start(
                        out=ktB[:, :, r],
                        in_=blk[:, 512:].rearrange("p (q c) -> p q c", c=P),
                    )
            vA = vnat[:, :32, :].rearrange("p (q rb) d -> p q rb d", rb=8)
            vB = vnat[:, 32:, :].rearrange("p (q rb) d -> p q rb d", rb=8)
            for r in range(NCORES):
                blk = cc_v_out[r * P : (r + 1) * P, :]
                eng = nc.sync if r % 2 == 0 else nc.scalar
                eng.dma_start(
                    out=vA[:, :, r],
                    in_=blk[:, :512].rearrange("p (q d) -> p q d", d=DV),
                )
                eng = nc.scalar if r % 2 == 0 else nc.sync
                eng.dma_start(
                    out=vB[:, :, r],
                    in_=blk[:, 512:].rearrange("p (q d) -> p q d", d=DV),
                )

            # ---------- phase C: attention ----------
            with (
                tc.tile_pool(name="abuf", bufs=2) as apool,
                tc.tile_pool(name="ebuf", bufs=2) as epool,
                tc.tile_pool(name="stats", bufs=8) as stats,
                tc.tile_pool(name="atp", bufs=4) as atpool,
                tc.tile_pool(name="osb", bufs=2) as osb_pool,
                tc.tile_pool(name="sps", bufs=4, space="PSUM") as spsum,
                tc.tile_pool(name="tps", bufs=2, space="PSUM") as tpsum,
                tc.tile_pool(name="ops", bufs=2, space="PSUM") as opsum,
            ):
                prev = None  # (slot, C, A_sb) pending PV

                def emit_pv(slot, C, a_sb):
                    o_ps = opsum.tile([P, DV], f32, tag="ops")
                    nj = 4 * C
                    for j0 in range(0, nj, 4):
                        tps = tpsum.tile([P, 4, P], bf16, tag="tps")
                        for jj in range(4):
                            nc.tensor.transpose(
                                tps[:, jj, :],
                                a_sb[:, bass.ts(j0 + jj, P)],
                                ident[:],
                            )
                        at = atpool.tile([P, 4, P], bf16, tag="at")
                        nc.vector.tensor_copy(at[:], tps[:])
                        for jj in range(4):
                            nc.tensor.matmul(
                                o_ps[:],
                                at[:, jj, :],
                                vnat[:, j0 + jj, :],
                                start=(j0 + jj == 0),
                                stop=(j0 + jj == nj - 1),
                            )
                    o_sb = osb_pool.tile([P, DV], f32, tag="osb")
                    nc.vector.tensor_scalar_mul(
                        o_sb[:], o_ps[:], rinv[:, slot : slot + 1]
                    )
                    nc.sync.dma_start(
                        out=out[bass.ts(POS[slot], P), :], in_=o_sb[:]
                    )

                for slot in range(NSLOT):
                    C = C_SLOT[slot]
                    buf = apool.tile([P, 16 * 512], f32, tag="buf")
                    a_sb = epool.tile([P, 16 * 512], bf16, tag="asb")
                    cmax = stats.tile([P, 16], f32, tag="cmax")
                    qsl = bass.ts(POS[slot], P)
                    for n in range(C):
                        sps = spsum.tile([P, 512], f32, tag="sps")
                        ksl = bass.ts(n, 512)
                        nc.tensor.matmul(
                            sps[:], q_hi[:, qsl], kt_hi[:, ksl],
                            start=True, stop=False,
                        )
                        nc.tensor.matmul(
                            sps[:], q_hi[:, qsl], kt_lo[:, ksl],
                            start=False, stop=False,
                        )
                        nc.tensor.matmul(
                            sps[:], q_lo[:, qsl], kt_hi[:, ksl],
                            start=False, stop=True,
                        )
                        if n >= C - 2:
                            nc.vector.tensor_add(
                                buf[:, bass.ts(n, 512)],
                                sps[:],
                                mask_sb[:, slot, n - (C - 2), :],
                            )
                            nc.vector.tensor_reduce(
                                cmax[:, n : n + 1],
                                buf[:, bass.ts(n, 512)],
                                axis=AX.X,
                                op=OP.max,
                            )
                        else:
                            nc.scalar.activation(
                                out=buf[:, bass.ts(n, 512)],
                                in_=sps[:],
                                func=ACT.Copy,
                            )
                            nc.vector.tensor_reduce(
                                cmax[:, n : n + 1],
                                sps[:],
                                axis=AX.X,
                                op=OP.max,
                            )
                    rmax = stats.tile([P, 1], f32, tag="rmax")
                    nc.vector.tensor_reduce(
                        rmax[:], cmax[:, :C], axis=AX.X, op=OP.max
                    )
                    negb = stats.tile([P, 1], f32, tag="negb")
                    nc.vector.tensor_scalar_mul(negb[:], rmax[:], -SCALE)
                    lsum = stats.tile([P, 1], f32, tag="lsum")
                    nc.scalar.activation(
                        out=a_sb[:, : C * 512],
                        in_=buf[:, : C * 512],
                        func=ACT.Exp,
                        bias=negb[:],
                        scale=SCALE,
                        accum_out=lsum[:],
                    )
                    nc.vector.reciprocal(rinv[:, slot : slot + 1], lsum[:])

                    if prev is not None:
                        emit_pv(*prev)
                    prev = (slot, C, a_sb)
                emit_pv(*prev)

    nc.compile()
    return nc


_NC_CACHE = None


def _get_nc():
    global _NC_CACHE
    if _NC_CACHE is None:
        _NC_CACHE = _build_nc()
    return _NC_CACHE


def _make_masks(c):
    """Additive causal masks for the last two score chunks of each slot,
    in global column-POSITION order (see _pos_to_g)."""
    m = np.zeros((NSLOT, 2, P, 512), dtype=np.float32)  # cast to bf16 at the end
    idx512 = np.arange(512)
    rows128 = np.arange(P)[:, None]
    for s in range(NSLOT):
        g = _slot_to_g(c, s)
        C = C_SLOT[s]
        rows = g * P + rows128
        for jj in range(2):
            idx = (C - 2 + jj) * 512 + idx512
            kc = np.array(
                [_pos_to_g(i // P) * P + (i % P) for i in idx]
            )[None, :]
            m[s, jj] = np.where(kc <= rows, 0.0, NEG)
    import ml_dtypes
    return m.astype(ml_dtypes.bfloat16)


def _prep_in_maps(x, w_q, w_k, w_v):
    x = np.ascontiguousarray(x, dtype=np.float32)
    _whi = []
    _wlo = []
    for w in (w_q, w_k, w_v):
        w = np.ascontiguousarray(w, dtype=np.float32)
        hi = w.astype(np.float16)
        _whi.append(hi)
        _wlo.append((w - hi.astype(np.float32)).astype(np.float16))
    in_maps = []
    for c in range(NCORES):
        rows = np.concatenate(
            [
                np.arange(
                    _slot_to_g(c, s) * P, (_slot_to_g(c, s) + 1) * P
                )
                for s in PORDER
            ]
        )
        xt = x[rows].T.reshape(16, P, MB).transpose(1, 0, 2)
        xt = np.ascontiguousarray(xt)
        xt_hi = xt.astype(np.float16)
        xt_lo = (xt - xt_hi.astype(np.float32)).astype(np.float16)
        in_maps.append(
            {
                "xt_hi": xt_hi,
                "xt_lo": xt_lo,
                "mask": _make_masks(c),
                "wq_hi": _whi[0], "wq_lo": _wlo[0],
                "wk_hi": _whi[1], "wk_lo": _wlo[1],
                "wv_hi": _whi[2],
            }
        )
    return in_maps


def _run(x, w_q, w_k, w_v, trace=False, trace_cores=None):
    nc = _get_nc()
    in_maps = _prep_in_maps(x, w_q, w_k, w_v)
    res = run_bass_kernel_spmd(
        nc,
        in_maps,
        list(range(NCORES)),
        trace=trace,
        trace_cores=trace_cores,
    )
    out = np.zeros((S, DV), dtype=np.float32)
    for c in range(NCORES):
        oc = res.results[c]["out"]
        for j, s in enumerate(PORDER):
            g = _slot_to_g(c, s)
            out[g * P : (g + 1) * P] = oc[j * P : (j + 1) * P]
    return out, res


def kernel(**inputs):
    out, _ = _run(inputs["x"], inputs["w_q"], inputs["w_k"], inputs["w_v"])
    return out
